# revision 9
# baseline (speedup 1.0000x reference)
"""Trainium2 Bass kernel for nn_AttentionEvaluatorModel (batch-data-parallel, 8 cores).

Model (per batch b):
  q = mapper(query, Wq, bq); f = mapper(features, Wf, bf); v = mapper(values, Wv, bv)
  attn = sigmoid(q @ f.T);  w = attn * ftw * mask
  pooled = w @ v;  h = mapper(pooled, Wc, bc);  out = h @ Wout + bout
where mapper layer: x = relu(x @ W + b) + x  ==  max(x @ (W + I) + b, x).

Sharding: pure DP over batch (B=32 -> 4 batches/core, 8 cores, no collectives).

v2 design notes:
  - All inputs are host-prepped: features/values pre-cast to bf16 (identical
    numerics to the previous on-chip SWDGE cast, half the HBM bytes), weights
    packed as W+I in stationary [ei, eo] layout, query/ftw/mask pre-laid-out.
  - features/values are loaded TRANSPOSED straight from DRAM through the DMA
    xbar (dma_start_transpose with a DRAM source): no staging copy, no
    separate SBUF->SBUF transpose pass.
  - mapper epilogue x_out = max(psum + b, x_in) cycles across DVE-STT,
    Pool-STT, and ACT(copy+bias) + max-on-DVE/Pool to balance engine load.
  - attention computed in [f-part, q] layout (f2 chunks stationary); pooled
    uses w-chunks as (tiny) stationary with v2-natural as moving rhs
    ([16, 256] PSUM out, 32 matmuls/batch).
  - v2 is xbar-transposed back to natural ([f-part, e]) for pooling; pooled
    output is xbar-transposed once per graph for the tiny c-mapper + head.
"""

from contextlib import ExitStack

import numpy as np

import concourse.bass as bass
import concourse.mybir as mybir
import concourse.tile as tile
from concourse import bacc

B, Q, F, E, NL, L = 32, 16, 4096, 256, 8, 2
NCORES = 8
BPC = B // NCORES          # batches per core = 4
RQ = BPC * Q               # rows for q/c mappers = 64
P = 128
EH = E // P                # e-halves = 2
OC = F // P                # 32 f-chunks of 128
NCH = 8                    # 512-col chunks per mapper layer half

F32 = mybir.dt.float32
BF16 = mybir.dt.bfloat16

AF = mybir.ActivationFunctionType
ALU = mybir.AluOpType

MATS = ("q", "f", "v", "c")


def build_nc(repeats=1):
    """Build the per-core Bass graph (same graph for all 8 cores, SPMD)."""
    nc = bacc.Bacc("TRN2", target_bir_lowering=False, debug=False,
                   num_devices=NCORES)

    d_feat = nc.dram_tensor("features", [BPC, F, E], BF16, kind="ExternalInput").ap()
    d_vals = nc.dram_tensor("values", [BPC, F, E], BF16, kind="ExternalInput").ap()
    d_ftw = nc.dram_tensor("ftwT", [BPC, P, OC], BF16, kind="ExternalInput").ap()
    d_msk = nc.dram_tensor("maskT", [BPC, P, OC], BF16, kind="ExternalInput").ap()
    d_qT = nc.dram_tensor("qT", [P, EH, RQ], BF16, kind="ExternalInput").ap()
    d_Wall = nc.dram_tensor("Wall", [P, 4, L, EH, E], BF16, kind="ExternalInput").ap()
    d_Wnall = nc.dram_tensor("Wnall", [P, 4, L, EH, E], BF16, kind="ExternalInput").ap()
    d_ball = nc.dram_tensor("ballT", [P, EH, 4, L], BF16, kind="ExternalInput").ap()
    d_wout = nc.dram_tensor("woutT", [P, EH, NL], BF16, kind="ExternalInput").ap()
    d_bout = nc.dram_tensor("boutT", [1, NL], BF16, kind="ExternalInput").ap()
    d_out = nc.dram_tensor("out", [BPC, Q, NL], F32, kind="ExternalOutput").ap()

    with tile.TileContext(nc) as tc:
        with ExitStack() as ctx:
            _emit(ctx, tc, nc, d_feat, d_vals, d_ftw, d_msk, d_qT,
                  d_Wall, d_Wnall, d_ball, d_wout, d_bout, d_out,
                  repeats=repeats)

    nc.compile()
    return nc


def _emit(ctx, tc, nc, d_feat, d_vals, d_ftw, d_msk, d_qT,
          d_Wall, d_Wnall, d_ball, d_wout, d_bout, d_out, repeats=1):
    consts = ctx.enter_context(tc.tile_pool(name="consts", bufs=1))
    xbuf = ctx.enter_context(tc.tile_pool(name="xbuf", bufs=1))
    small = ctx.enter_context(tc.tile_pool(name="small", bufs=2))
    zps = ctx.enter_context(tc.tile_pool(name="zps", bufs=6, space="PSUM"))
    pps = ctx.enter_context(tc.tile_pool(name="ppsum", bufs=2, space="PSUM"))

    # ---------------- constants (host-prepped layouts, plain loads) --------
    Wall = consts.tile([P, 4, L, EH, E], BF16, tag="Wall")
    nc.sync.dma_start(Wall[:], d_Wall)
    Wnall = consts.tile([P, 4, L, EH, E], BF16, tag="Wnall")
    nc.sync.dma_start(Wnall[:], d_Wnall)
    ballT = consts.tile([P, EH, 4, L], BF16, tag="ballT")
    nc.sync.dma_start(ballT[:], d_ball)
    qT = consts.tile([P, EH, RQ], BF16, tag="qT0")
    nc.sync.dma_start(qT[:], d_qT)
    woutb = consts.tile([P, EH, NL], BF16, tag="woutb")
    nc.sync.dma_start(woutb[:], d_wout)
    boutb = consts.tile([1, NL], BF16, tag="boutb")
    nc.sync.dma_start(boutb[:], d_bout)
    ones_row = consts.tile([1, RQ], BF16, tag="ones_row")
    nc.vector.memset(ones_row[:], 1.0)

    def w_ap(mi, l, ei, eo):
        return Wall[:, mi, l, ei, eo * P:(eo + 1) * P]

    def wn_ap(mi, l, ei, eo):
        return Wnall[:, mi, l, ei, eo * P:(eo + 1) * P]

    def b_ap(mi, l, eo):
        return ballT[:, eo, mi, l:l + 1]

    # ---------------- input prefetch ----------------
    loaded = {}

    def prefetch(b):
        if b >= BPC or b in loaded:
            return
        fxT = xbuf.tile([P, EH, F], BF16, tag="fxT", bufs=2, name="fxT")
        nc.sync.dma_start_transpose(fxT[:], d_feat[b])
        vxT = xbuf.tile([P, EH, F], BF16, tag="vxT", bufs=2, name="vxT")
        nc.sync.dma_start_transpose(vxT[:], d_vals[b])
        ftw_t = small.tile([P, OC], BF16, tag="ftw", name="ftw")
        nc.sync.dma_start(ftw_t[:], d_ftw[b])
        msk_t = small.tile([P, OC], BF16, tag="msk", name="msk")
        nc.sync.dma_start(msk_t[:], d_msk[b])
        s_t = small.tile([P, OC], BF16, tag="s", name="s")
        nc.vector.tensor_tensor(s_t[:], ftw_t[:], msk_t[:], ALU.mult)
        loaded[b] = (s_t, fxT, vxT)

    prefetch(0)

    # ---------------- epilogue: x_out = relu(z + b) + x_in ----------------
    # PSUM can only be read by DVE/ACT (GpSimd has no PSUM access), and Pool
    # tensor_tensor only supports add/mult, so:
    # A: DVE scalar_tensor_tensor max (1 op, W+I psum)
    # B: ACT relu(psum + b) -> t, then t + x_in on DVE (SBUF bf16, plain W)
    # C: same as B with the add on Pool
    ep_count = [0]
    FORM_CYCLE = "AC"

    def next_form():
        u = ep_count[0]
        ep_count[0] += 1
        return FORM_CYCLE[u % len(FORM_CYCLE)]

    def epilogue(form, zpsum, bias, x_in, x_out):
        if form == "A":
            nc.vector.scalar_tensor_tensor(
                out=x_out, in0=zpsum, scalar=bias, in1=x_in,
                op0=ALU.add, op1=ALU.max)
        else:
            t = small.tile(list(zpsum.shape), BF16, tag="eptmp", bufs=6,
                           name="eptmp")
            nc.scalar.activation(t[:], zpsum, AF.Relu, bias=bias)
            if form == "B":
                nc.vector.tensor_tensor(x_out, t[:], x_in, ALU.add)
            else:
                nc.gpsimd.tensor_tensor(x_out, t[:], x_in, ALU.add)

    # ---------------- q-mapper (tiny) ----------------
    def small_mapper(xT, mi, mat):
        """xT [128, EH, RQ] bf16 -> mapper output, same layout."""
        cur = xT
        for l in range(L):
            nxt = consts.tile([P, EH, RQ], BF16, tag=f"{mat}T{l + 1}",
                              name=f"{mat}T{l + 1}")
            for eo in range(EH):
                ps = zps.tile([P, RQ], F32, tag="zpsum", name="qcpsum")
                for ei in range(EH):
                    nc.tensor.matmul(ps[:], lhsT=w_ap(mi, l, ei, eo),
                                     rhs=cur[:, ei, :],
                                     start=(ei == 0), stop=(ei == EH - 1))
                epilogue("A", ps[:], b_ap(mi, l, eo), cur[:, eo, :],
                         nxt[:, eo, :])
            cur = nxt
        return cur

    q2T = small_mapper(qT, 0, "q")

    # ---------------- big mapper ----------------
    def big_mapper(xT, mi):
        cur = xT
        for l in range(L):
            nxt = xbuf.tile([P, EH, F], BF16, tag=f"xl{l + 1}", bufs=2,
                            name=f"xl{l + 1}")
            for eo in range(EH):
                for c in range(NCH):
                    sl = slice(512 * c, 512 * (c + 1))
                    form = next_form()
                    wsel = w_ap if form == "A" else wn_ap
                    ps = zps.tile([P, 512], F32, tag="zpsum", name="zpsum")
                    for ei in range(EH):
                        nc.tensor.matmul(ps[:], lhsT=wsel(mi, l, ei, eo),
                                         rhs=cur[:, ei, sl],
                                         start=(ei == 0), stop=(ei == EH - 1))
                    epilogue(form, ps[:], b_ap(mi, l, eo),
                             cur[:, eo, sl], nxt[:, eo, sl])
            cur = nxt
        return cur

    # ---------------- per-batch pipeline ----------------
    # [16(q), BPC, 256(e)] so every per-batch drain starts at partition 0
    pooled_nat = consts.tile([Q, BPC, E], BF16, tag="pooled_nat")

    for rep in range(repeats):
      if rep > 0:
          loaded.clear()
          prefetch(0)
      for b in range(BPC):
        s_t, fxT, vxT = loaded.pop(b)

        # ---- f path
        f2 = big_mapper(fxT, 1)

        # ---- next batch's loads overlap this batch's compute
        prefetch(b + 1)

        # ---- attention logits -> sigmoid -> w   [f-part, chunk, q]
        aps = zps.tile([P, OC, Q], F32, tag="zpsum", name="attnps")
        for c in range(OC):
            for h in range(EH):
                nc.tensor.matmul(aps[:, c, :], lhsT=f2[:, h, c * P:(c + 1) * P],
                                 rhs=q2T[:, h, b * Q:(b + 1) * Q],
                                 start=(h == 0), stop=(h == EH - 1))
        att_b = small.tile([P, OC, Q], BF16, tag="attnsb", name="attnsb")
        nc.scalar.activation(att_b[:], aps[:], AF.Sigmoid)
        w_t = small.tile([P, OC, Q], BF16, tag="w", name="w")
        nc.gpsimd.tensor_tensor(w_t[:], att_b[:],
                                s_t[:, :, None].to_broadcast((P, OC, Q)),
                                ALU.mult)

        # ---- v path
        v2 = big_mapper(vxT, 2)

        # ---- v2 back to natural: v2n[p, m, 128h+r] = v2[f=128m+p, e=128h+r]
        v2n = xbuf.tile([P, OC, E], BF16, tag="v2n", bufs=2, name="v2n")
        for h in range(EH):
            nc.sync.dma_start_transpose(v2n[:, :, h * P:(h + 1) * P],
                                        v2[:, h, :])

        # ---- pooled[q, e] accumulated over f-chunks (w-chunk stationary)
        pps_t = pps.tile([Q, E], F32, tag="poolps", name="poolps")
        for j in range(OC):
            nc.tensor.matmul(pps_t[:], lhsT=w_t[:, j, :], rhs=v2n[:, j, :],
                             start=(j == 0), stop=(j == OC - 1))
        nc.vector.tensor_copy(pooled_nat[:, b, :], pps_t[:])

    # ---------------- pooled -> c-mapper -> head ----------------
    # transpose [16, (b e)] -> [128, 2*BPC(m = 2b+h), 16(q)]:
    # pooledT[p, m, r] = pooled[b = m//2][q = r][e = 128*(m%2) + p]
    pooledT = consts.tile([P, EH * BPC, Q], BF16, tag="pooledT")
    nc.sync.dma_start_transpose(pooledT[:],
                                pooled_nat.rearrange("q b e -> q (b e)"))

    def c_in(t, l, h):
        # layer-0/1 input layout [P, 2b+h, q]; final layout [P, h, b, q]
        return t[:, h::2, :] if l < L else t[:, h, :, :]

    cur = pooledT
    for l in range(L):
        last = l == L - 1
        nxt = consts.tile([P, EH, BPC, Q] if last else [P, EH * BPC, Q],
                          BF16, tag=f"cT{l + 1}", name=f"cT{l + 1}")
        for eo in range(EH):
            ps = zps.tile([P, BPC, Q], F32, tag="zpsum", name="cpsum")
            for ei in range(EH):
                nc.tensor.matmul(ps[:], lhsT=w_ap(3, l, ei, eo),
                                 rhs=c_in(cur, l, ei),
                                 start=(ei == 0), stop=(ei == EH - 1))
            epilogue("A", ps[:], b_ap(3, l, eo), c_in(cur, l, eo),
                     c_in(nxt, l + 1, eo) if last else nxt[:, eo::2, :])
        cur = nxt
    h2T = cur

    out_ps = zps.tile([RQ, NL], F32, tag="zpsum", name="outps")
    for h in range(EH):
        nc.tensor.matmul(out_ps[:], lhsT=h2T[:, h, :, :], rhs=woutb[:, h, :],
                         start=(h == 0), stop=False)
    nc.tensor.matmul(out_ps[:], lhsT=ones_row[:], rhs=boutb[:],
                     start=False, stop=True)
    out_sb = small.tile([RQ, NL], F32, tag="outsb")
    nc.vector.tensor_copy(out_sb[:], out_ps[:])
    nc.sync.dma_start(d_out.rearrange("b q n -> (b q) n"), out_sb[:])


def make_in_maps(inputs):
    """Host-side prep: shard over batch, cast to bf16, pack weight layouts."""
    import ml_dtypes
    bf16 = ml_dtypes.bfloat16

    ident = np.eye(E, dtype=np.float32)
    # Wall[p, mi, l, ei, eo] = (W[m][l] + I)[ei*128+p, eo]
    Wp = np.stack([np.asarray(inputs[f"W{m}"], np.float32) + ident
                   for m in MATS])                      # [4, L, 256, 256]
    Wall = np.ascontiguousarray(
        Wp.reshape(4, L, EH, P, E).transpose(3, 0, 1, 2, 4).astype(bf16))
    Wn = np.stack([np.asarray(inputs[f"W{m}"], np.float32) for m in MATS])
    Wnall = np.ascontiguousarray(
        Wn.reshape(4, L, EH, P, E).transpose(3, 0, 1, 2, 4).astype(bf16))
    # ballT[p, eo, mi, l] = b[m][l][eo*128+p]
    ball = np.stack([np.asarray(inputs[f"b{m}"], np.float32)
                     for m in MATS])                    # [4, L, 256]
    ballT = np.ascontiguousarray(
        ball.reshape(4, L, EH, P).transpose(3, 2, 0, 1).astype(bf16))
    woutT = np.ascontiguousarray(
        np.asarray(inputs["Wout"], np.float32)
        .reshape(EH, P, NL).transpose(1, 0, 2).astype(bf16))
    boutT = np.asarray(inputs["bout"], np.float32).reshape(1, NL).astype(bf16)

    query = np.asarray(inputs["query"], np.float32)      # [B, Q, E]
    feats = np.asarray(inputs["features"], np.float32).astype(bf16)
    vals = np.asarray(inputs["values"], np.float32).astype(bf16)
    ftw = np.asarray(inputs["feature_time_weights"], np.float32)
    msk = np.asarray(inputs["attention_mask"], np.float32)

    in_maps = []
    for c in range(NCORES):
        sl = slice(c * BPC, (c + 1) * BPC)
        q_c = query[sl]                                  # [BPC, Q, E]
        qT = np.ascontiguousarray(
            q_c.reshape(BPC, Q, EH, P).transpose(3, 2, 0, 1)
            .reshape(P, EH, RQ).astype(bf16))
        # ftwT[b, p, oc] = ftw[b, 128*oc + p]
        ftwT = np.ascontiguousarray(
            ftw[sl].reshape(BPC, OC, P).transpose(0, 2, 1).astype(bf16))
        mskT = np.ascontiguousarray(
            msk[sl].reshape(BPC, OC, P).transpose(0, 2, 1).astype(bf16))
        in_maps.append({
            "features": np.ascontiguousarray(feats[sl]),
            "values": np.ascontiguousarray(vals[sl]),
            "ftwT": ftwT,
            "maskT": mskT,
            "qT": qT,
            "Wall": Wall,
            "Wnall": Wnall,
            "ballT": ballT,
            "woutT": woutT,
            "boutT": boutT,
        })
    return in_maps


_NC_CACHE = {}


def get_nc():
    if "nc" not in _NC_CACHE:
        _NC_CACHE["nc"] = build_nc()
    return _NC_CACHE["nc"]


def kernel(**inputs) -> np.ndarray:
    from concourse.bass_utils import run_bass_kernel_spmd

    inputs = {k: np.asarray(v) for k, v in inputs.items()}
    nc = get_nc()
    in_maps = make_in_maps(inputs)
    res = run_bass_kernel_spmd(nc, in_maps, core_ids=list(range(NCORES)))
    out = np.concatenate([res.results[c]["out"] for c in range(NCORES)], axis=0)
    return out.astype(np.float32)
